# revision 35
# baseline (speedup 1.0000x reference)
"""Trainium2 Bass kernel for an 8-sequence transformer block.

Reference computation (per sequence l of L=8, data-parallel over 8 cores):
  qkv = x @ qkv_w ; split q,k,v ; 4 heads x 32 dims
  attn = softmax(q @ k^T / sqrt(32)) @ v          (mask is all-ones)
  h    = LN(attn @ out_w + x)
  ff   = relu(relu(h @ w1 + b1) @ w2 + b2)
  out  = LN(ff + h)

v2 strategy: everything on-chip, transposed layout [feature(part), seq(free)],
bf16 matmuls.  Scores run 4-way row-tiled (K=32 per head, tile_position
(32h,0)) so all 4 heads' score matmuls execute concurrently in the PE array.
Context runs 4-way col-tiled (M=32, tile_position (0,32h)); softmax
denominators come from 4 extra col-tiled M=1 ones-matmuls into a dedicated
psum bank.  The exp of the 16.8M scores is split across two engines: ACT
computes true exp for ~60%, DVE computes a Schraudolph-style approximate
exp for the rest with a single tensor_scalar (fp32 psum -> int16 bits that
reinterpret as bf16).  PSUM budget: scores 2+2 banks, ctx 1, den 1, tail 2.
"""

import sys
import types
from contextlib import ExitStack

import numpy as np

import bass_rust
import concourse.bass as bass
import concourse.tile as tile
from concourse import mybir
from concourse.bass_utils import run_bass_kernel_spmd
from concourse.vector_clock import ScopedClock

# ---------------------------------------------------------------------------
# Workaround: this walrus build rejects >1 sem waits on the TileContext tail
# drain ("Too many sync wait commands").  Redistribute the drain's waits onto
# single-wait SP nop carriers.
# ---------------------------------------------------------------------------


def _patched_drain_and_barrier(self, tick_clock, wait_clock):
    nc = self.nc
    drain_inst = nc.sync.drain()
    wait_clock.add_sem_waits(
        drain_inst.ins, ScopedClock({None: tick_clock.global_clock})
    )
    inst = drain_inst.ins
    waits = list(inst.sync_info.on_wait)
    if len(waits) > 1:
        inst.sync_info.on_wait = waits[:1]
        for w in waits[1:]:
            n = nc.sync.nop(nofuse=True, hint="drain_wait_carrier")
            n.ins.sync_info = bass_rust.SyncInfo(on_wait=[w], on_update=[])

    nc.all_engine_barrier()
    assert self.sems is not None
    popped = nc._tile_sem_poison_stack.pop()
    assert popped is self._sem_poison
    nc.clear_and_free_semaphores(list(self.sems.allocated().values()))
    nc.all_engine_barrier()


tile.TileContext._drain_and_barrier = _patched_drain_and_barrier

# ---------------------------------------------------------------------------
# Workaround #2: this walrus build allows only ONE sem wait per instruction
# on several instruction structs (Matmult/Drain/...).  Post-process the BIR
# JSON before compile: keep the last wait on the instruction and move the
# rest onto same-engine NoOp carriers inserted right before it.
# ---------------------------------------------------------------------------

import json as _json

import concourse.bass2jax as _bass2jax
import concourse.bass_utils as _bass_utils

_orig_compile_bir_kernel = _bass_utils.compile_bir_kernel


def _split_excess_waits(bir_json):
    if isinstance(bir_json, (bytes, bytearray)):
        d = _json.loads(bir_json.decode())
    else:
        d = _json.loads(bir_json)
    nid = 0
    changed = False
    for fn in d["functions"]:
        for blk in fn["blocks"]:
            new_insts = []
            for inst in blk["instructions"]:
                si = inst.get("sync_info")
                waits = (si or {}).get("on_wait") or []
                if len(waits) > 1:
                    changed = True
                    for w in waits[:-1]:
                        nid += 1
                        new_insts.append({
                            "name": f"I-wsplit-{nid}",
                            "opcode": "NoOp",
                            "engine": inst["engine"],
                            "ins": [],
                            "outs": [],
                            "sync_info": {"on_wait": [w], "on_update": []},
                            "text_hint": "wait_split",
                        })
                    si["on_wait"] = waits[-1:]
                new_insts.append(inst)
            blk["instructions"] = new_insts
    if not changed:
        return bir_json
    return _json.dumps(d).encode()


def _patched_compile_bir_kernel(bir_json, tmpdir, neff_name="file.neff", **kw):
    return _orig_compile_bir_kernel(
        _split_excess_waits(bir_json), tmpdir, neff_name=neff_name, **kw)


_bass_utils.compile_bir_kernel = _patched_compile_bir_kernel
_bass2jax.compile_bir_kernel = _patched_compile_bir_kernel

# ---------------------------------------------------------------------------

L, S, D = 8, 2048, 128
H, HD = 4, 32
FH = 384
NCHUNK = S // 128          # 16 k chunks of 128
NQ = S // 512              # 4 q chunks of 512
SCALE = 1.0 / np.sqrt(HD)
LN_EPS = 1e-5
F32 = mybir.dt.float32
BF16 = mybir.dt.bfloat16
I16 = mybir.dt.int16
EXP = mybir.ActivationFunctionType.Exp
LN_F = mybir.ActivationFunctionType.Ln
COPY_F = mybir.ActivationFunctionType.Copy
ADD = mybir.AluOpType.add
SUB = mybir.AluOpType.subtract
MULT = mybir.AluOpType.mult
MAXOP = mybir.AluOpType.max

# Schraudolph exp-as-bf16-bits: bits = round(x * SCH_A + SCH_B), bitcast bf16
SCH_A = 128.0 * 1.4426950408889634
SCH_B = 127.0 * 128.0 - 7.5

# exp engine assignment: the A half (heads 0,1) always goes to ACT; the B
# half (heads 2,3) goes to DVE except every 4th unit, rebalancing load.
def _b_half_on_act(u):
    return u % 4 == 1


def _build_nc():
    nc = bass.Bass("TRN2", target_bir_lowering=False, debug=False)

    dram = {}
    for name, shape in (
        ("x", [S, D]), ("qkv_w", [D, 3 * D]), ("out_w", [D, D]),
        ("w1", [D, FH]), ("w2", [FH, D]), ("b1", [FH]), ("b2", [D]),
        ("g1", [D]), ("be1", [D]), ("g2", [D]), ("be2", [D]),
        ("ident", [128, 128]), ("sel4", [128, 128]),
    ):
        dram[name] = nc.dram_tensor(name, shape, F32, kind="ExternalInput").ap()
    dram["out"] = nc.dram_tensor("out", [S, D], F32, kind="ExternalOutput").ap()

    with tile.TileContext(nc) as tc:
        _emit(nc, tc, dram)
    return nc


def _emit(nc, tc, dram):
    ctx = ExitStack()
    with ctx:
        consts = ctx.enter_context(tc.tile_pool(name="consts", bufs=1))
        acts = ctx.enter_context(tc.tile_pool(name="acts", bufs=1))
        wstage = ctx.enter_context(tc.tile_pool(name="wstage", bufs=1))

        # --- tiny dummy exp first so the ACT table loads during the DMAs ---
        dummy = consts.tile([1, 8], F32, tag="dummy", name="dummy")
        nc.gpsimd.memset(dummy[:], 0.0)
        nc.scalar.activation(dummy[:], dummy[:], EXP)

        # ---- stage fp32 inputs ----
        x_sb = wstage.tile([128, NCHUNK, 128], F32, tag="x_sb", name="x_sb")
        x_src = dram["x"].rearrange("(n p) d -> p n d", p=128)
        x_qs = (nc.sync, nc.scalar, nc.sync, nc.scalar)
        for g in range(4):
            x_qs[g].dma_start(x_sb[:, 4 * g:4 * (g + 1), :],
                              x_src[:, 4 * g:4 * (g + 1), :])

        def stage(name, shape, src_ap, engine="gpsimd"):
            t = wstage.tile(shape, F32, tag=name + "_s", name=name + "_s")
            getattr(nc, engine).dma_start(t[:], src_ap)
            return t

        ident_s = stage("ident", [128, 128], dram["ident"][:], "sync")
        sel4_s = stage("sel4", [128, 128], dram["sel4"][:])
        wqkv_s = stage("wqkv", [D, 3 * D], dram["qkv_w"][:], "scalar")
        wout_s = stage("wout", [D, D], dram["out_w"][:])
        w1_s = stage("w1", [D, FH], dram["w1"][:])
        w2_s = stage("w2", [128, 3, 128],
                     dram["w2"].rearrange("(c p) d -> p c d", p=128))

        def cast_bf(src, tag, pool=consts):
            t = pool.tile(list(src.shape), BF16, tag=tag, name=tag)
            nc.vector.tensor_copy(t[:], src[:])
            return t

        identb = cast_bf(ident_s, "identb")
        sel4r = consts.tile([128, 128], mybir.dt.float32r, tag="sel4r",
                            name="sel4r")
        nc.vector.tensor_copy(sel4r[:], sel4_s[:])
        wqkvb = cast_bf(wqkv_s, "wqkvb")
        woutb = cast_bf(wout_s, "woutb")
        w1b = cast_bf(w1_s, "w1b")
        w2b = cast_bf(w2_s, "w2b")

        b1c = consts.tile([128, 3], F32, tag="b1c", name="b1c")
        nc.gpsimd.dma_start(b1c[:], dram["b1"].rearrange("(c p) -> p c", p=128))
        cols = {}
        for name in ("b2", "g1", "be1", "g2", "be2"):
            t = consts.tile([128, 1], F32, tag=name + "c", name=name + "c")
            nc.gpsimd.dma_start(t[:], dram[name].rearrange("(p o) -> p o", o=1))
            cols[name] = t
        jmean = consts.tile([128, 128], BF16, tag="jmean", name="jmean")
        nc.gpsimd.memset(jmean[:], 1.0 / 128.0)
        ones_col = consts.tile([128, 1], BF16, tag="ones_col", name="ones_col")
        nc.gpsimd.memset(ones_col[:], 1.0)
        eps_col = consts.tile([128, 1], F32, tag="eps_col", name="eps_col")
        nc.gpsimd.memset(eps_col[:], LN_EPS)
        # zero stationary/moving rows for psum-bank zeroing matmuls
        zcol = consts.tile([1, 128], BF16, tag="zcol", name="zcol")
        nc.gpsimd.memset(zcol[:], 0.0)
        zrow = consts.tile([1, 512], BF16, tag="zrow", name="zrow")
        nc.gpsimd.memset(zrow[:], 0.0)

        # ---- x -> bf16, transpose to xt [d, s] ----
        x_bf = wstage.tile([128, NCHUNK, 128], BF16, tag="x_bf", name="x_bf")
        nc.vector.tensor_copy(x_bf[:], x_sb[:])
        xt = acts.tile([128, S], BF16, tag="xt", name="xt")
        qt = acts.tile([128, S], BF16, tag="qt", name="qt")
        kt = acts.tile([128, S], BF16, tag="kt", name="kt")
        v_sb = acts.tile([128, NCHUNK, 128], BF16, tag="v_sb", name="v_sb")
        et_pool = ctx.enter_context(tc.tile_pool(name="et_pool", bufs=4))
        out_sb = acts.tile([128, NCHUNK, 128], F32, tag="out_sb", name="out_sb")

        with tc.tile_pool(name="ps_pre", bufs=2, space="PSUM") as ps_pre:
            for g in range(4):
                pt = ps_pre.tile([128, 512], BF16, tag="pt", bufs=2, name="pt")
                for u in range(4):
                    n = 4 * g + u
                    nc.tensor.transpose(pt[:, u * 128:(u + 1) * 128],
                                        x_bf[:, n, :], identb[:])
                nc.vector.tensor_copy(xt[:, g * 512:(g + 1) * 512], pt[:])
            # k then q projections: [f, s] layout; kt j=0 and qt j=0 first
            # so the first score matmuls can start while the rest project.
            for dst, m, j in ((kt, 1, 0), (qt, 0, 0),
                              (kt, 1, 1), (kt, 1, 2), (kt, 1, 3),
                              (qt, 0, 1), (qt, 0, 2), (qt, 0, 3)):
                pq = ps_pre.tile([128, 512], F32, tag="pq", bufs=2,
                                 name="pq")
                nc.tensor.matmul(pq[:], wqkvb[:, m * 128:(m + 1) * 128],
                                 xt[:, j * 512:(j + 1) * 512],
                                 start=True, stop=True)
                nc.scalar.activation(dst[:, j * 512:(j + 1) * 512], pq[:],
                                     COPY_F)
            # v in [kpos, f] layout
            for g in range(4):
                pv = ps_pre.tile([128, 512], F32, tag="pv", bufs=2, name="pv")
                for u in range(4):
                    n = 4 * g + u
                    nc.tensor.matmul(pv[:, u * 128:(u + 1) * 128],
                                     xt[:, n * 128:(n + 1) * 128],
                                     wqkvb[:, 256:384], start=True, stop=True)
                nc.vector.tensor_copy(v_sb[:, 4 * g:4 * (g + 1), :], pv[:])

        # ---- attention + tail, pipelined per q chunk of 512 ----
        with (
            tc.tile_pool(name="ps_sA", bufs=1, space="PSUM") as ps_sA,
            tc.tile_pool(name="ps_sB", bufs=1, space="PSUM") as ps_sB,
            tc.tile_pool(name="ps_ctx", bufs=1, space="PSUM") as ps_ctx,
            tc.tile_pool(name="ps_den", bufs=1, space="PSUM") as ps_den,
            tc.tile_pool(name="ps_tail", bufs=1, space="PSUM") as ps_tail,
            tc.tile_pool(name="ck", bufs=2) as ck,
        ):
            den_ps = ps_den.tile([128, 512], F32, tag="den", bufs=1,
                                 name="den")
            nc.vector.memset(den_ps[:], 0.0)
            for qc in range(NQ):
                _qchunk(nc, tc, qc, ps_sA, ps_sB, ps_ctx, den_ps, ps_tail,
                        et_pool, ck, qt, kt, v_sb, xt, out_sb, dram,
                        identb, sel4r, jmean, ones_col, woutb, w1b, w2b,
                        b1c, cols, eps_col, zcol, zrow)


def _qchunk(nc, tc, qc, ps_sA, ps_sB, ps_ctx, den_ps, ps_tail,
            et_pool, ck, qt, kt, v_sb, xt, out_sb, dram,
            identb, sel4r, jmean, ones_col, woutb, w1b, w2b, b1c, cols,
            eps_col, zcol, zrow):
    qs = slice(qc * 512, (qc + 1) * 512)
    ctx_ps = ps_ctx.tile([128, 512], F32, tag="ctx", bufs=1, name="ctx")
    # zero both accumulator banks with a single K=1 matmul each; the per-head
    # accumulation matmuls then all run with start=False so no mid-stream
    # has_written clear can wipe another head's partial sums.
    nc.tensor.matmul(ctx_ps[:], zcol[:], zrow[:], start=True, stop=True,
                     skip_group_check=True)
    nc.tensor.matmul(den_ps[:], zcol[:], zrow[:], start=True, stop=True,
                     skip_group_check=True)

    ets = {}
    # score psum: [128, 1024] = 2 banks per half-pair; heads of a pair sit
    # in different banks (write concurrency), and successive 256-wide units
    # alternate column parity within the banks -> effective double buffering
    # inside the same 4-bank footprint.
    sA = ps_sA.tile([128, 1024], F32, tag="sA", bufs=1, name="sA")
    sB = ps_sB.tile([128, 1024], F32, tag="sB", bufs=1, name="sB")
    sAv = sA.rearrange("p (b c) -> p b c", b=2)
    sBv = sB.rearrange("p (b c) -> p b c", b=2)

    def emit_scores(u):
        kc, sub = u // 2, u % 2
        p = u % 2
        ps_ = slice(p * 256, p * 256 + 256)
        uqs = slice(qc * 512 + sub * 256, qc * 512 + sub * 256 + 256)
        for h in range(H):
            dst = sA if h < 2 else sB
            nc.tensor.matmul(
                dst[:, (h % 2) * 512 + p * 256:(h % 2) * 512 + p * 256 + 256],
                kt[32 * h:32 * h + 32, kc * 128:(kc + 1) * 128],
                qt[32 * h:32 * h + 32, uqs],
                start=True, stop=True, tile_position=(32 * h, 0))
        et = et_pool.tile([128, H, 256], BF16, tag="et", name="et")
        # exp: A half on ACT (true exp), B half on DVE (Schraudolph bits)
        nc.scalar.activation(et[:, 0:2, :], sAv[:, :, ps_], EXP,
                             scale=float(SCALE))
        if _b_half_on_act(u):
            nc.scalar.activation(et[:, 2:4, :], sBv[:, :, ps_], EXP,
                                 scale=float(SCALE))
        else:
            et_i16 = et.bitcast(I16)
            nc.vector.tensor_scalar(
                et_i16[:, 2:4, :], sBv[:, :, ps_],
                float(SCH_A * SCALE), float(SCH_B), op0=MULT, op1=ADD)
        ets[u] = et

    def emit_ctx(u):
        kc, sub = u // 2, u % 2
        cs = slice(sub * 256, sub * 256 + 256)
        et = ets.pop(u)
        last = u >= 2 * NCHUNK - 2
        for h in range(H):
            nc.tensor.matmul(
                ctx_ps[32 * h:32 * h + 32, cs],
                v_sb[:, kc, 32 * h:32 * h + 32], et[:, h, :],
                start=False, stop=last,
                tile_position=(0, 32 * h), skip_group_check=True)
        for h in range(H):
            nc.tensor.matmul(
                den_ps[32 * h:32 * h + 1, cs],
                ones_col[:], et[:, h, :],
                start=False, stop=last,
                tile_position=(0, 32 * h), skip_group_check=True)

    NU = 2 * NCHUNK
    emit_scores(0)
    for u in range(1, NU):
        emit_scores(u)
        emit_ctx(u - 1)
    emit_ctx(NU - 1)

    # softmax denominator -> log, broadcast per head block, exp(-x) = 1/den
    lden = ck.tile([128, 512], mybir.dt.float32r, tag="lden", name="lden")
    nc.scalar.activation(lden[:], den_ps[:], LN_F, bias=eps_col[:])
    # tail: the last q chunk's tail is on the critical path with nothing to
    # overlap, so run it as two 256-wide halves to shorten the serial chain.
    if qc == NQ - 1:
        for half in range(2):
            _tail(nc, ps_tail, ck, qc, half * 256, 256, ctx_ps, lden,
                  qt, kt, xt, out_sb, dram, identb, sel4r, jmean,
                  woutb, w1b, w2b, b1c, cols, eps_col)
    else:
        _tail(nc, ps_tail, ck, qc, 0, 512, ctx_ps, lden, qt, kt, xt,
              out_sb, dram, identb, sel4r, jmean, woutb, w1b, w2b, b1c,
              cols, eps_col)


def _tail(nc, ps_tail, ck, qc, o0, w, ctx_ps, lden, qt, kt, xt, out_sb,
          dram, identb, sel4r, jmean, woutb, w1b, w2b, b1c, cols, eps_col):
    q0 = qc * 512 + o0
    qs = slice(q0, q0 + w)
    os_ = slice(o0, o0 + w)
    lbc = ps_tail.tile([128, 512], F32, tag="pt0", bufs=2, name="lbc")[:, 0:w]
    nc.tensor.matmul(lbc[:], sel4r[:], lden[:, os_], start=True, stop=True)
    rden = ck.tile([128, 512], F32, tag="rden", name="rden")[:, 0:w]
    nc.scalar.activation(rden[:], lbc[:], EXP, scale=-1.0)
    atile = ck.tile([128, 512], BF16, tag="atile", name="atile")[:, 0:w]
    nc.vector.tensor_tensor(atile[:], ctx_ps[:, os_], rden[:], op=MULT)

    po = ps_tail.tile([128, 512], F32, tag="pt0", bufs=2, name="po")[:, 0:w]
    nc.tensor.matmul(po[:], woutb[:], atile[:], start=True, stop=True)
    h1 = ck.tile([128, 512], BF16, tag="h1", name="h1")[:, 0:w]
    nc.vector.tensor_tensor(h1[:], po[:], xt[:, qs], op=ADD)

    h1n = ck.tile([128, 512], BF16, tag="h1n", name="h1n")[:, 0:w]
    _layernorm(nc, ps_tail, ck, h1, h1n, jmean, cols["g1"], cols["be1"], "1",
               eps_col, w)

    ff1 = ck.tile([128, 3, 512], BF16, tag="ff1", name="ff1")[:, :, 0:w]
    for c in range(3):
        pf = ps_tail.tile([128, 512], F32, tag="pt0", bufs=2,
                          name="pf")[:, 0:w]
        nc.tensor.matmul(pf[:], w1b[:, c * 128:(c + 1) * 128], h1n[:],
                         start=True, stop=True)
        nc.vector.tensor_scalar(ff1[:, c, :], pf[:], b1c[:, c:c + 1], 0.0,
                                op0=ADD, op1=MAXOP)
    pf2 = ps_tail.tile([128, 512], F32, tag="pt0", bufs=2, name="pf2")[:, 0:w]
    for c in range(3):
        nc.tensor.matmul(pf2[:], w2b[:, c, :], ff1[:, c, :],
                         start=(c == 0), stop=(c == 2))
    tmp = ck.tile([128, 512], BF16, tag="ff2t", name="ff2t")[:, 0:w]
    nc.vector.tensor_scalar(tmp[:], pf2[:], cols["b2"][:], 0.0,
                            op0=ADD, op1=MAXOP)
    h2 = ck.tile([128, 512], BF16, tag="h2", name="h2")[:, 0:w]
    nc.vector.tensor_tensor(h2[:], tmp[:], h1n[:], op=ADD)

    outt = ck.tile([128, 512], BF16, tag="outt", name="outt")[:, 0:w]
    _layernorm(nc, ps_tail, ck, h2, outt, jmean, cols["g2"], cols["be2"], "2",
               eps_col, w)

    ot = ps_tail.tile([128, 512], BF16, tag="pt0", bufs=2, name="ot")[:, 0:w]
    for u in range(w // 128):
        nc.tensor.transpose(ot[:, u * 128:(u + 1) * 128],
                            outt[:, u * 128:(u + 1) * 128], identb[:])
    n0 = q0 // 128
    nc.vector.tensor_copy(out_sb[:, n0:n0 + w // 128, :],
                          ot.rearrange("p (n d) -> p n d", n=w // 128))
    nc.sync.dma_start(
        dram["out"].rearrange("(n p) d -> p n d", p=128)[
            :, n0:n0 + w // 128, :],
        out_sb[:, n0:n0 + w // 128, :])


def _layernorm(nc, ps_tail, ck, src, dst, jmean, g_col, be_col, sfx,
               eps_col, w):
    """dst = g * (src - mean) / sqrt(var + eps) + be over the partition
    (feature) axis.  jmean matmul broadcasts the mean; var = mean((x-m)^2)
    via a second jmean matmul; rstd = exp(-0.5*ln(var+eps))."""
    pm = ps_tail.tile([128, 512], F32, tag="pt0", bufs=2,
                      name="pm" + sfx)[:, 0:w]
    nc.tensor.matmul(pm[:], jmean[:], src[:], start=True, stop=True)
    xmm = ck.tile([128, 512], BF16, tag="xmm" + sfx, name="xmm" + sfx)[:, 0:w]
    nc.vector.scalar_tensor_tensor(xmm[:], src[:], 1.0, pm[:],
                                   op0=MULT, op1=SUB)
    sq = ck.tile([128, 512], BF16, tag="sq" + sfx, name="sq" + sfx)[:, 0:w]
    nc.vector.tensor_tensor(sq[:], xmm[:], xmm[:], op=MULT)
    pv = ps_tail.tile([128, 512], F32, tag="pt0", bufs=2,
                      name="pv" + sfx)[:, 0:w]
    nc.tensor.matmul(pv[:], jmean[:], sq[:], start=True, stop=True)
    lnv = ck.tile([128, 512], F32, tag="lnv" + sfx, name="lnv" + sfx)[:, 0:w]
    nc.scalar.activation(lnv[:], pv[:], LN_F, bias=eps_col[:])
    rstd = ck.tile([128, 512], BF16, tag="rstd" + sfx,
                   name="rstd" + sfx)[:, 0:w]
    nc.scalar.activation(rstd[:], lnv[:], EXP, scale=-0.5)
    t = ck.tile([128, 512], BF16, tag="lnt" + sfx, name="lnt" + sfx)[:, 0:w]
    nc.vector.tensor_tensor(t[:], xmm[:], rstd[:], op=MULT)
    nc.vector.tensor_scalar(dst[:], t[:], g_col[:], be_col[:],
                            op0=MULT, op1=ADD)


_NC = None


def _get_nc():
    global _NC
    if _NC is None:
        _NC = _build_nc()
    return _NC


def _make_in_maps(inputs):
    x = np.ascontiguousarray(np.asarray(inputs["x"], dtype=np.float32))
    shared = {
        k: np.ascontiguousarray(np.asarray(inputs[k], dtype=np.float32))
        for k in ("qkv_w", "out_w", "w1", "w2", "b1", "b2",
                  "g1", "be1", "g2", "be2")
    }
    shared["ident"] = np.eye(128, dtype=np.float32)
    # sel4[p, m] = 1 iff p == 32*(m//32): broadcast head denominators
    sel4 = np.zeros((128, 128), dtype=np.float32)
    for m in range(128):
        sel4[32 * (m // 32), m] = 1.0
    shared["sel4"] = sel4
    return [dict(shared, x=x[l]) for l in range(L)]


def kernel(**inputs):
    nc = _get_nc()
    in_maps = _make_in_maps(inputs)
    res = run_bass_kernel_spmd(nc, in_maps, core_ids=list(range(L)))
    return np.stack([res.results[l]["out"] for l in range(L)], axis=0)


def run_with_trace(inputs, tmpdir):
    """Used by test.py: same as kernel() but captures an NTFF profile."""
    _register_ntff_hook()
    nc = _get_nc()
    in_maps = _make_in_maps(inputs)
    res = run_bass_kernel_spmd(nc, in_maps, core_ids=list(range(L)),
                               trace=True, tmpdir=tmpdir)
    out = np.stack([res.results[l]["out"] for l in range(L)], axis=0)
    return out, res


def _register_ntff_hook():
    try:
        from antenv.axon_hooks import get_axon_ntff_profile_hook  # noqa: F401
        return
    except ImportError:
        pass
    mod = types.ModuleType("antenv.axon_hooks")
    mod._hook = None

    def set_axon_ntff_profile_hook(h):
        mod._hook = h

    def get_axon_ntff_profile_hook():
        return mod._hook

    mod.set_axon_ntff_profile_hook = set_axon_ntff_profile_hook
    mod.get_axon_ntff_profile_hook = get_axon_ntff_profile_hook
    import antenv
    sys.modules["antenv.axon_hooks"] = mod
    antenv.axon_hooks = mod
    from trn_agent_boot.trn_boot import _ntff_profile_via_ctypes
    set_axon_ntff_profile_hook(_ntff_profile_via_ctypes("/opt/axon/libaxon_pjrt.so"))
